# revision 10
# baseline (speedup 1.0000x reference)
"""Trainium2 Bass kernel for nn_Attention_81458349736162.

Batch-parallel over the 8 NeuronCores: each core owns B/8 = 4 batches and
runs the full attention + MLP for them; no collectives are needed.

Per batch b (N=4096 defects, H=512):
  ua   = Ua @ normal_b + Ua_b                  (tiny matmuls)
  c    = Wa_b - ua                             [H]
  For each 128-defect tile:
    load d [128, 512] f32, cast bf16, PE-transpose -> dT [h, n]
    PSUM group: z = c (seed rank-1 matmul) + sum_h WaT_h^T... i.e.
        diff[n, o] = sum_h d[n,h] Wa[o,h] + c[o]
    dist2[n] = sum_o diff^2 via ScalarE Square + accum_out
  dist = exp(0.5*ln(dist2))  (ln+exp share one ACT table set; sqrt does not)
  e    = exp(dist - 20)      (constant-shift softmax; shift cancels exactly)
  S    = sum(e);  context = (sum_n e_n d_n) / S   (PE matmuls on resident bf16 d)
  out  = W2 @ relu(W1 @ [context, global] + b1) + b2   (f32, tiny)
"""

import os
import numpy as np

B, N, H, OUT, MID = 32, 4096, 512, 5, 128
NCORES = 8
BLOC = B // NCORES          # batches per core
P = 128                     # partitions
T = N // P                  # 32 n-tiles per batch
HC = H // P                 # 4 h-chunks
MB = 2048                   # free-dim elems per DMA group (4 tiles of 512)
G = (T * H) // MB           # 8 DMA groups per batch
SHIFT = 20.0                # softmax shift constant (dist ~ 18.5 +- 1)

_CACHE = {}


def _build_program():
    import concourse.tile as tile
    import concourse.mybir as mybir
    from concourse import bacc
    from contextlib import ExitStack

    f32 = mybir.dt.float32
    bf16 = mybir.dt.bfloat16
    AF = mybir.ActivationFunctionType
    ALU = mybir.AluOpType

    nc = bacc.Bacc("TRN2", target_bir_lowering=False, debug=False,
                   num_devices=NCORES)

    # ---- DRAM I/O (per-core shards; weights replicated) ----
    defect = nc.dram_tensor("defect_embeddings", [BLOC * N, H], f32,
                            kind="ExternalInput").ap()
    normal = nc.dram_tensor("normal_embedding", [BLOC, H], f32,
                            kind="ExternalInput").ap()
    glob = nc.dram_tensor("global_features", [BLOC, H], f32,
                          kind="ExternalInput").ap()
    wa_w = nc.dram_tensor("Wa_w", [H, H], f32, kind="ExternalInput").ap()
    wa_b = nc.dram_tensor("Wa_b", [1, H], f32, kind="ExternalInput").ap()
    ua_w = nc.dram_tensor("Ua_w", [H, H], f32, kind="ExternalInput").ap()
    ua_b = nc.dram_tensor("Ua_b", [1, H], f32, kind="ExternalInput").ap()
    w1 = nc.dram_tensor("W1", [MID, 2 * H], f32, kind="ExternalInput").ap()
    b1 = nc.dram_tensor("b1", [1, MID], f32, kind="ExternalInput").ap()
    w2 = nc.dram_tensor("W2", [OUT, MID], f32, kind="ExternalInput").ap()
    b2 = nc.dram_tensor("b2", [1, OUT], f32, kind="ExternalInput").ap()
    out_d = nc.dram_tensor("out", [1, BLOC * OUT], f32,
                           kind="ExternalOutput").ap()

    with tile.TileContext(nc, num_cores=NCORES) as tc, ExitStack() as ctx:
        consts = ctx.enter_context(tc.tile_pool(name="consts", bufs=1))
        wload = ctx.enter_context(tc.tile_pool(name="wload", bufs=1))
        dstream = ctx.enter_context(tc.tile_pool(name="dstream", bufs=3))
        dbatch = ctx.enter_context(tc.tile_pool(name="dbatch", bufs=2))
        dtp = ctx.enter_context(tc.tile_pool(name="dtp", bufs=3))
        sqs = ctx.enter_context(tc.tile_pool(name="sqs", bufs=2))
        bstat = ctx.enter_context(tc.tile_pool(name="bstat", bufs=2))
        ps_tp = ctx.enter_context(tc.tile_pool(name="ps_tp", bufs=2, space="PSUM"))
        ps_dist = ctx.enter_context(tc.tile_pool(name="ps_dist", bufs=2, space="PSUM"))
        ps_ctx = ctx.enter_context(tc.tile_pool(name="ps_ctx", bufs=1, space="PSUM"))
        ps_small = ctx.enter_context(tc.tile_pool(name="ps_small", bufs=2, space="PSUM"))

        # ---------------- constants ----------------
        ones_f32 = consts.tile([P, P], f32)
        nc.vector.memset(ones_f32[:], 1.0)
        ident_f32 = consts.tile([P, P], f32)
        nc.gpsimd.affine_select(ident_f32[:], ones_f32[:], pattern=[[-1, P]],
                                compare_op=ALU.is_equal, fill=0.0, base=0,
                                channel_multiplier=1)
        ones_bf = consts.tile([P, P], bf16)
        nc.vector.memset(ones_bf[:], 1.0)
        neg_shift_col = consts.tile([P, 1], f32)
        nc.vector.memset(neg_shift_col[:], -SHIFT)
        ident_bf = consts.tile([P, P], bf16)
        nc.gpsimd.affine_select(ident_bf[:], ones_bf[:], pattern=[[-1, P]],
                                compare_op=ALU.is_equal, fill=0.0, base=0,
                                channel_multiplier=1)

        def transpose_128(dst_sb, src_sb, nblk, ident, psum_pool, dt):
            """Transpose nblk [128,128] blocks src->dst (through PSUM)."""
            tp = psum_pool.tile([P, nblk * P], dt, tag="tp_ps")
            for i in range(nblk):
                nc.tensor.transpose(tp[:, i * P:(i + 1) * P],
                                    src_sb[:, i * P:(i + 1) * P], ident[:])
            nc.vector.tensor_copy(dst_sb[:], tp[:])

        # ---------------- weights: load + transpose ----------------
        # WaT_bf / UaT_bf layout: [128 (h_low), hc*512 + o]
        wat_bf = consts.tile([P, HC * H], bf16)
        uat_bf = consts.tile([P, HC * H], bf16)
        for (wdram, wtile) in ((wa_w, wat_bf), (ua_w, uat_bf)):
            wnat_f = wload.tile([P, HC * H], f32, tag="wnat_f")
            for oc in range(HC):
                nc.sync.dma_start(wnat_f[:, oc * H:(oc + 1) * H],
                                  wdram[oc * P:(oc + 1) * P, :])
            wnat_b = wload.tile([P, HC * H], bf16, tag="wnat_b")
            nc.vector.tensor_copy(wnat_b[:], wnat_f[:])
            # transpose 16 blocks: block (oc, hc) -> (hc, oc)
            for hc in range(HC):
                tp = ps_tp.tile([P, H], bf16, tag="tp_ps")
                for oc in range(HC):
                    nc.tensor.transpose(
                        tp[:, oc * P:(oc + 1) * P],
                        wnat_b[:, oc * H + hc * P: oc * H + (hc + 1) * P],
                        ident_bf[:])
                nc.vector.tensor_copy(wtile[:, hc * H:(hc + 1) * H], tp[:])

        # W1T: [128 (feat_low), fc*128 + m] ; feature f = fc*128 + feat_low
        w1t = consts.tile([P, 2 * H], f32)
        w1nat = wload.tile([MID, 2 * H], f32, tag="w1nat")
        nc.sync.dma_start(w1nat[:], w1[:])
        for fc in range(2 * H // P):
            tp = ps_small.tile([P, P], f32, tag="sm_ps")
            nc.tensor.transpose(tp[:, :],
                                w1nat[:, fc * P:(fc + 1) * P], ident_f32[:])
            nc.vector.tensor_copy(w1t[:, fc * P:(fc + 1) * P], tp[:])

        # W2T [128, 5]
        w2t = consts.tile([P, OUT], f32)
        w2nat = wload.tile([OUT, MID], f32, tag="w2nat")
        nc.sync.dma_start(w2nat[:], w2[:])
        tp = ps_small.tile([P, OUT], f32, tag="sm_ps")
        nc.tensor.transpose(tp[:, :], w2nat[:, :], ident_f32[:OUT, :OUT])
        nc.vector.tensor_copy(w2t[:], tp[:])

        # bias rows/cols
        b1_col = consts.tile([P, 1], f32)
        b1row = wload.tile([1, MID], f32, tag="b1row")
        nc.sync.dma_start(b1row[:], b1[:])
        tp = ps_small.tile([P, 1], f32, tag="sm_ps")
        nc.tensor.transpose(tp[:, :], b1row[:, :], ident_f32[:1, :1])
        nc.vector.tensor_copy(b1_col[:], tp[:])

        b2_row = consts.tile([1, OUT], f32)
        nc.sync.dma_start(b2_row[:], b2[:])

        wab_row = consts.tile([1, H], f32)
        nc.sync.dma_start(wab_row[:], wa_b[:])
        uab_row = consts.tile([1, H], f32)
        nc.sync.dma_start(uab_row[:], ua_b[:])
        wb_minus_ub = consts.tile([1, H], f32)
        nc.vector.tensor_sub(wb_minus_ub[:], wab_row[:], uab_row[:])

        # global features, transposed per batch: globT [128, b*4 + fc]
        globT = consts.tile([P, BLOC * HC], f32)
        gnat = wload.tile([1, BLOC * H], f32, tag="gnat")
        nc.sync.dma_start(gnat[:], glob[:].rearrange("b h -> () (b h)"))
        for b in range(BLOC):
            tp = ps_small.tile([P, HC], f32, tag="sm_ps")
            for fc in range(HC):
                nc.tensor.transpose(
                    tp[:, fc:fc + 1],
                    gnat[:1, b * H + fc * P: b * H + (fc + 1) * P],
                    ident_f32[:1, :1])
            nc.vector.tensor_copy(globT[:, b * HC:(b + 1) * HC], tp[:])

        # normal embeddings, bf16 transposed: normT [128, b*4 + hc]
        normT = consts.tile([P, BLOC * HC], bf16)
        normT_f = consts.tile([P, BLOC * HC], f32)
        nnat = wload.tile([1, BLOC * H], f32, tag="gnat2")
        nc.sync.dma_start(nnat[:], normal[:].rearrange("b h -> () (b h)"))
        for b in range(BLOC):
            tp = ps_small.tile([P, HC], f32, tag="sm_ps")
            for hc in range(HC):
                nc.tensor.transpose(
                    tp[:, hc:hc + 1],
                    nnat[:1, b * H + hc * P: b * H + (hc + 1) * P],
                    ident_f32[:1, :1])
            nc.vector.tensor_copy(normT_f[:, b * HC:(b + 1) * HC], tp[:])
        nc.vector.tensor_copy(normT[:], normT_f[:])

        result_sb = consts.tile([1, BLOC * OUT], f32)

        # ---------------- per-batch main loop ----------------
        for b in range(BLOC):
            # ua = Ua @ normal_b + Ua_b ; c = Wa_b - ua
            ua_ps = ps_small.tile([1, H], f32, tag="sm_ps")
            for hc in range(HC):
                nc.tensor.matmul(ua_ps[:, :],
                                 normT[:, b * HC + hc: b * HC + hc + 1],
                                 uat_bf[:, hc * H:(hc + 1) * H],
                                 start=(hc == 0), stop=(hc == HC - 1))
            c_row = bstat.tile([1, H], f32, tag="c_row")
            nc.vector.tensor_sub(c_row[:], wb_minus_ub[:], ua_ps[:])
            c_bf = bstat.tile([1, H], bf16, tag="c_bf")
            nc.vector.tensor_copy(c_bf[:], c_row[:])

            d_bf = dbatch.tile([P, T * H], bf16, tag="d_bf")
            dist2 = bstat.tile([P, T], f32, tag="dist2")

            for g in range(G):
                dmb = dstream.tile([P, MB], f32, tag="dmb")
                nc.sync.dma_start(
                    dmb[:],
                    defect[b * N + g * (MB // H) * P:
                           b * N + (g + 1) * (MB // H) * P, :]
                    .rearrange("(a p) h -> p a h", p=P))
                nc.vector.tensor_copy(d_bf[:, g * MB:(g + 1) * MB], dmb[:])

                for ti in range(MB // H):
                    t = g * (MB // H) + ti
                    # transpose d tile -> dT [h, n]
                    tp = ps_tp.tile([P, H], bf16, tag="tp_ps")
                    for hc in range(HC):
                        nc.tensor.transpose(
                            tp[:, hc * P:(hc + 1) * P],
                            d_bf[:, t * H + hc * P: t * H + (hc + 1) * P],
                            ident_bf[:])
                    dT = dtp.tile([P, H], bf16, tag="dT")
                    nc.vector.tensor_copy(dT[:], tp[:])

                    # diff[n, o] = c[o] + sum_h d[n,h] Wa[o,h]
                    dist_ps = ps_dist.tile([P, H], f32, tag="dist_ps")
                    nc.tensor.matmul(dist_ps[:, :], ones_bf[:1, :],
                                     c_bf[:, :], start=True, stop=False)
                    for hc in range(HC):
                        nc.tensor.matmul(dist_ps[:, :],
                                         dT[:, hc * P:(hc + 1) * P],
                                         wat_bf[:, hc * H:(hc + 1) * H],
                                         start=False, stop=(hc == HC - 1))
                    # dist2[:, t] = sum_o diff^2
                    sq = sqs.tile([P, H], bf16, tag="sq")
                    nc.scalar.activation(sq[:], dist_ps[:], AF.Square,
                                         accum_out=dist2[:, t:t + 1])

            # ---- softmax stats (constant shift, no cross-tile max) ----
            tln = bstat.tile([P, T], f32, tag="tln")
            nc.scalar.activation(tln[:], dist2[:], AF.Ln)
            dist_sb = bstat.tile([P, T], f32, tag="dist_sb")
            nc.scalar.activation(dist_sb[:], tln[:], AF.Exp, scale=0.5)
            e_f32 = bstat.tile([P, T], f32, tag="e_f32")
            nc.scalar.activation(e_f32[:], dist_sb[:], AF.Exp,
                                 bias=neg_shift_col[:])
            e_bf = bstat.tile([P, T], bf16, tag="e_bf")
            nc.vector.tensor_copy(e_bf[:], e_f32[:])

            # S = sum(e)
            e_rs = bstat.tile([P, 1], f32, tag="e_rs")
            nc.vector.reduce_sum(e_rs[:], e_f32[:], axis=mybir.AxisListType.X)
            tp = ps_small.tile([1, P], f32, tag="sm_ps")
            nc.tensor.transpose(tp[:, :], e_rs[:, :], ident_f32[:])
            srow = bstat.tile([1, P], f32, tag="srow")
            nc.vector.tensor_copy(srow[:], tp[:])
            s_sc = bstat.tile([1, 1], f32, tag="s_sc")
            nc.vector.reduce_sum(s_sc[:], srow[:], axis=mybir.AxisListType.X)
            recip_s = bstat.tile([1, 1], f32, tag="recip_s")
            nc.vector.reciprocal(recip_s[:], s_sc[:])

            # ---- context = (sum_n e_n d_n) / S ----
            ctx_ps = ps_ctx.tile([1, H], f32, tag="ctx_ps")
            for t in range(T):
                nc.tensor.matmul(ctx_ps[:, :], e_bf[:, t:t + 1],
                                 d_bf[:, t * H:(t + 1) * H],
                                 start=(t == 0), stop=(t == T - 1))
            context_sb = bstat.tile([1, H], f32, tag="context_sb")
            nc.scalar.activation(context_sb[:], ctx_ps[:], AF.Copy,
                                 scale=recip_s[:1, :1])

            # ---- MLP ----
            tp = ps_small.tile([P, HC], f32, tag="sm_ps")
            for fc in range(HC):
                nc.tensor.transpose(tp[:, fc:fc + 1],
                                    context_sb[:, fc * P:(fc + 1) * P],
                                    ident_f32[:1, :1])
            combT = bstat.tile([P, HC], f32, tag="combT")
            nc.vector.tensor_copy(combT[:], tp[:])

            h1_ps = ps_small.tile([P, 1], f32, tag="sm_ps")
            for fc in range(2 * H // P):
                rhs = (combT[:, fc:fc + 1] if fc < HC
                       else globT[:, b * HC + fc - HC: b * HC + fc - HC + 1])
                nc.tensor.matmul(h1_ps[:, :], w1t[:, fc * P:(fc + 1) * P],
                                 rhs, start=(fc == 0),
                                 stop=(fc == 2 * H // P - 1))
            h1_sb = bstat.tile([P, 1], f32, tag="h1_sb")
            nc.scalar.activation(h1_sb[:], h1_ps[:], AF.Relu, bias=b1_col[:])

            o_ps = ps_small.tile([1, OUT], f32, tag="sm_ps")
            nc.tensor.matmul(o_ps[:, :], h1_sb[:, :], w2t[:, :],
                             start=True, stop=True)
            nc.vector.tensor_add(result_sb[:, b * OUT:(b + 1) * OUT],
                                 o_ps[:], b2_row[:])

        nc.sync.dma_start(out_d[:], result_sb[:])

    nc.compile()
    return nc


def _get_program():
    if "nc" not in _CACHE:
        _CACHE["nc"] = _build_program()
    return _CACHE["nc"]


def _make_in_maps(inputs):
    fp = np.float32
    d = np.ascontiguousarray(inputs["defect_embeddings"], dtype=fp)
    nrm = np.ascontiguousarray(inputs["normal_embedding"], dtype=fp)
    gf = np.ascontiguousarray(inputs["global_features"], dtype=fp)
    shared = {
        "Wa_w": np.ascontiguousarray(inputs["Wa_w"], dtype=fp),
        "Wa_b": np.ascontiguousarray(inputs["Wa_b"], dtype=fp).reshape(1, H),
        "Ua_w": np.ascontiguousarray(inputs["Ua_w"], dtype=fp),
        "Ua_b": np.ascontiguousarray(inputs["Ua_b"], dtype=fp).reshape(1, H),
        "W1": np.ascontiguousarray(inputs["W1"], dtype=fp),
        "b1": np.ascontiguousarray(inputs["b1"], dtype=fp).reshape(1, MID),
        "W2": np.ascontiguousarray(inputs["W2"], dtype=fp),
        "b2": np.ascontiguousarray(inputs["b2"], dtype=fp).reshape(1, OUT),
    }
    in_maps = []
    for c in range(NCORES):
        lo, hi = c * BLOC, (c + 1) * BLOC
        m = dict(shared)
        m["defect_embeddings"] = np.ascontiguousarray(
            d[lo:hi].reshape(BLOC * N, H))
        m["normal_embedding"] = np.ascontiguousarray(
            nrm[lo:hi].reshape(BLOC, H))
        m["global_features"] = np.ascontiguousarray(gf[lo:hi])
        in_maps.append(m)
    return in_maps


def _install_ntff_hook_shim():
    """The agent image's antenv package lacks axon_hooks; recreate it so
    run_bass_kernel_spmd(trace=True) can capture NTFF profiles."""
    import sys
    import types

    try:
        from antenv.axon_hooks import get_axon_ntff_profile_hook  # noqa: F401
        return
    except ImportError:
        pass
    import antenv
    from trn_agent_boot import trn_boot

    so_path = "/opt/axon/libaxon_pjrt.so"
    hook = trn_boot._ntff_profile_via_ctypes(so_path)
    if hook is None:
        raise RuntimeError("libaxon_pjrt.so lacks profile symbols")
    mod = types.ModuleType("antenv.axon_hooks")
    state = {"hook": hook}
    mod.set_axon_ntff_profile_hook = lambda h: state.__setitem__("hook", h)
    mod.get_axon_ntff_profile_hook = lambda: state["hook"]
    sys.modules["antenv.axon_hooks"] = mod
    antenv.axon_hooks = mod


def kernel(**inputs) -> np.ndarray:
    from concourse.bass_utils import run_bass_kernel_spmd

    nc = _get_program()
    in_maps = _make_in_maps(inputs)
    trace = bool(int(os.environ.get("KERNEL_TRACE", "0")))
    if trace:
        try:
            _install_ntff_hook_shim()
        except Exception:
            trace = False
    res = run_bass_kernel_spmd(nc, in_maps, core_ids=list(range(NCORES)),
                               trace=trace)
    if res.exec_time_ns is not None:
        print(f"HW exec time: {res.exec_time_ns} ns")
    out = np.concatenate(
        [res.results[c]["out"].reshape(BLOC, OUT) for c in range(NCORES)],
        axis=0)
    return out.astype(np.float32)
